# revision 2
# baseline (speedup 1.0000x reference)
"""Multi-head attention (B=2, S=2048, D=768, H=16, dk=48) on 8 TRN2 NeuronCores.

Sharding: core c = (batch b = c//4, head-group g = c%4 of 4 heads).
Each core computes Q/K/V projections for its 4 heads, full attention over
S=2048, and a partial output projection (contribution of its heads).
Host sums the 4 partials per batch and adds the analytically-folded biases.

v2 restructure (vs baseline): the scalar engine's 128 Exp ops are the
bottleneck (~1.04us each); everything else is scheduled around keeping them
back-to-back:
- software pipelining: ctx matmuls for step k are emitted LAG=4 steps later,
  so scores(k+1) is never stuck in the PE FIFO behind a ctx waiting on exp(k);
- pair-major group order (all 4 quarters of head-pair 0, then pair 1) so the
  K/Q projection work for pair 1 has late deadlines;
- projections split into <=2-matmul filler parts placed greedily (by deadline)
  into per-step PE slack;
- softmax-normalize broadcast matmul deferred 2 steps so it never blocks the
  PE FIFO while waiting on the reciprocal;
- prologue-critical input DMAs (wk/wq/x-chunk0/wv) issued on the ACT HWDGE
  ring so they prefetch across loop iterations while the SP ring drains
  output DMAs.
"""
import os
import sys
from collections import deque

import numpy as np
import ml_dtypes

for _p in ("/opt/trn_rl_repo", "/opt/pypackages"):
    if os.path.isdir(_p) and _p not in sys.path:
        sys.path.append(_p)

import concourse.bacc as bacc
import concourse.mybir as mybir
import concourse.tile as tile
from concourse.bass_utils import run_bass_kernel_spmd

F32 = mybir.dt.float32
F32R = mybir.dt.float32r
BF16 = mybir.dt.bfloat16
NPBF16 = ml_dtypes.bfloat16

B = 2
S = 2048
D = 768
H = 16
DK = 48
HPC = 4            # heads per core
NPAIR = 2          # head pairs per core
E = NPAIR * 128    # padded per-core head dim (4 heads x 64)
KT = D // 128      # 6 contraction tiles for projections
ST = S // 128      # 16 s-tiles
NQ = 4             # sq quarters
QW = S // NQ       # 512
NCORES = 8
LAG = 4            # ctx matmuls trail scores/exp by this many steps

_PROGRAM = None


class _Task:
    __slots__ = ("ready", "deadline", "seq", "cost", "fn")

    def __init__(self, ready, deadline, seq, cost, fn):
        self.ready, self.deadline, self.seq = ready, deadline, seq
        self.cost, self.fn = cost, fn


def _build_program(variant="full"):
    nc = bacc.Bacc("TRN2", target_bir_lowering=False, debug=False)

    xT = nc.dram_tensor("xT", [D, S], BF16, kind="ExternalInput")
    wq = nc.dram_tensor("wq", [D, E], BF16, kind="ExternalInput")
    wk = nc.dram_tensor("wk", [D, E], BF16, kind="ExternalInput")
    wv = nc.dram_tensor("wv", [D, E], BF16, kind="ExternalInput")
    wo = nc.dram_tensor("wo", [E, D], BF16, kind="ExternalInput")
    bq = nc.dram_tensor("bq", [E], F32, kind="ExternalInput")
    bk = nc.dram_tensor("bk", [E], F32, kind="ExternalInput")
    ones2 = nc.dram_tensor("ones2", [2, 128], F32R, kind="ExternalInput")
    out = nc.dram_tensor("out", [S, D], F32, kind="ExternalOutput")

    EXPF = mybir.ActivationFunctionType.Exp

    with tile.TileContext(nc) as tc:
        with (
            tc.tile_pool(name="xw", bufs=1) as xw,          # x + weights
            tc.tile_pool(name="qkv", bufs=1) as qkv,        # qT/kT/v/ctxT
            tc.tile_pool(name="expp", bufs=8) as expp,      # exp tiles
            tc.tile_pool(name="outp", bufs=4) as outp,      # ctxu + out staging
            tc.tile_pool(name="misc", bufs=4) as misc,      # denom/recip
            tc.tile_pool(name="ps_sc", bufs=2, space="PSUM") as ps_sc,   # 4 banks
            tc.tile_pool(name="ps_ctx", bufs=1, space="PSUM") as ps_ctx,  # 2 banks
            tc.tile_pool(name="ps_aux", bufs=3, space="PSUM") as ps_aux,  # 3 banks
        ):
            # ---------- input DMAs ----------
            # Prologue-critical tensors go on the ACT HWDGE ring: dispatched at
            # body top (ACT idles there), so across For_i iterations they
            # prefetch while the SP ring is still draining output DMAs.
            xT_sb = [xw.tile([128, S], BF16, name=f"xT_sb{k}", tag=f"xT_sb{k}")
                     for k in range(KT)]
            w_sb = {nm: [xw.tile([128, E], BF16, name=f"{nm}_sb{k}",
                                 tag=f"{nm}_sb{k}") for k in range(KT)]
                    for nm in ("wk", "wq", "wv")}
            for k in range(KT):
                nc.sync.dma_start(out=w_sb["wk"][k][:],
                                    in_=wk[128 * k:128 * (k + 1), :])
                nc.sync.dma_start(out=xT_sb[k][:, 0:512],
                                    in_=xT[128 * k:128 * (k + 1), 0:512])
                nc.sync.dma_start(out=w_sb["wq"][k][:],
                                    in_=wq[128 * k:128 * (k + 1), :])
            bias_sb = {}
            for nm, dram in (("bq", bq), ("bk", bk)):
                t = xw.tile([128, NPAIR], F32, name=f"{nm}_sb", tag=f"{nm}_sb")
                nc.sync.dma_start(out=t[:], in_=dram.rearrange("(t p) -> p t", p=128))
                bias_sb[nm] = t
            for k in range(KT):
                nc.sync.dma_start(out=w_sb["wv"][k][:],
                                    in_=wv[128 * k:128 * (k + 1), :])

            # Bulk of x + output-projection weights on the SP ring.
            for c in range(1, 4):
                for k in range(KT):
                    nc.sync.dma_start(
                        out=xT_sb[k][:, 512 * c:512 * (c + 1)],
                        in_=xT[128 * k:128 * (k + 1), 512 * c:512 * (c + 1)])
            wo_sb = []
            for k in range(NPAIR):
                t = xw.tile([128, D], BF16, name=f"wo_sb{k}", tag=f"wo_sb{k}")
                nc.sync.dma_start(out=t[:], in_=wo[128 * k:128 * (k + 1), :])
                wo_sb.append(t)
            ones_sb = xw.tile([2, 128], F32R, name="ones_sb", tag="ones_sb")
            nc.sync.dma_start(out=ones_sb[:], in_=ones2[:])

            # ---------- persistent activations (bf16) ----------
            qT_sb = [qkv.tile([128, S], BF16, name=f"qT_sb{p}", tag=f"qT_sb{p}")
                     for p in range(NPAIR)]
            kT_sb = [qkv.tile([128, S], BF16, name=f"kT_sb{p}", tag=f"kT_sb{p}")
                     for p in range(NPAIR)]
            v_bf = [qkv.tile([128, E], BF16, name=f"v_bf{st}", tag=f"v_bf{st}")
                    for st in range(ST)]
            ctxT_sb = [qkv.tile([128, S], BF16, name=f"ctxT_sb{p}", tag=f"ctxT_sb{p}")
                       for p in range(NPAIR)]

            # ---------- projection part emitters ----------
            chunk_ps = {}

            def emit_qk_part(nm, t, c, phase):
                dst = kT_sb if nm == "wk" else qT_sb
                if phase == 0:
                    chunk_ps[(nm, t, c)] = ps_aux.tile(
                        [128, 512], F32, name=f"ps_{nm}{t}_{c}", tag="ps_aux")
                ps = chunk_ps[(nm, t, c)]
                for k in range(2 * phase, 2 * phase + 2):
                    nc.tensor.matmul(
                        ps[:],
                        lhsT=w_sb[nm][k][:, 128 * t:128 * (t + 1)],
                        rhs=xT_sb[k][:, 512 * c:512 * (c + 1)],
                        start=(k == 0), stop=(k == KT - 1),
                        skip_group_check=True,
                    )
                if phase == 2:
                    bias = "bk" if nm == "wk" else "bq"
                    with nc.allow_low_precision(reason="bf16 q/k"):
                        nc.vector.tensor_scalar_add(
                            dst[t][:, 512 * c:512 * (c + 1)], ps[:],
                            bias_sb[bias][:, t:t + 1])

            def emit_qk_chunk(nm, t, c):
                for ph in range(3):
                    emit_qk_part(nm, t, c, ph)

            def emit_v_part(st, phase):
                if phase == 0:
                    chunk_ps[("v", st)] = ps_aux.tile(
                        [128, 512], F32, name=f"ps_v{st}", tag="ps_aux")
                psv = chunk_ps[("v", st)][:, 0:E]
                for k in range(3 * phase, 3 * phase + 3):
                    nc.tensor.matmul(
                        psv,
                        lhsT=xT_sb[k][:, 128 * st:128 * (st + 1)],
                        rhs=w_sb["wv"][k][:],
                        start=(k == 0), stop=(k == KT - 1),
                        skip_group_check=True,
                    )
                if phase == 1:
                    with nc.allow_low_precision(reason="probs@v in bf16"):
                        for j in range(HPC):
                            nc.vector.tensor_copy(
                                v_bf[st][:, 64 * j:64 * j + DK],
                                psv[:, 64 * j:64 * j + DK])
                    for j in range(HPC):
                        nc.gpsimd.memset(v_bf[st][:, 64 * j + 48:64 * j + 49], 1.0)

            o_sb_tiles = {}

            def emit_outproj_chunk(q, sti, c0, c1):
                st = q * (QW // 128) + sti
                if st not in o_sb_tiles:
                    o_sb_tiles[st] = outp.tile([128, D], F32, name=f"o_sb{st}",
                                               tag="o_sb")
                o_sb = o_sb_tiles[st]
                ps = ps_aux.tile([128, 512], F32, name=f"ps_o{st}_{c0}",
                                 tag="ps_aux")
                pso = ps[:, 0:c1 - c0]
                for k in range(NPAIR):
                    nc.tensor.matmul(
                        pso,
                        lhsT=ctxT_sb[k][:, 128 * st:128 * (st + 1)],
                        rhs=wo_sb[k][:, c0:c1],
                        start=(k == 0), stop=(k == NPAIR - 1),
                    )
                nc.vector.tensor_copy(o_sb[:, c0:c1], pso)
                nc.sync.dma_start(out=out[128 * st:128 * (st + 1), c0:c1],
                                  in_=o_sb[:, c0:c1])

            if variant.startswith("scexp"):
                # Minimal scores->exp pipeline (junk data straight from xT):
                # scexp2 / scexp3 = sc ring of 2 / 3; scexpctx2 adds lagged
                # ctx matmuls like the real kernel.
                nring = int(variant[-1])
                with_ctx = "ctx" in variant
                scs, exs2 = [], []
                ctxp = None
                if with_ctx:
                    ctxp = [ps_ctx.tile([128, 512], F32, name=f"mbctx{s}",
                                        tag=f"ps_ctx{s}") for s in range(2)]
                for i in range(128):
                    sc = ps_sc.tile([128, 1024], F32, name=f"mbsc{i}",
                                    tag="ps_sc", bufs=nring)
                    for side in range(2):
                        r0 = 64 * side
                        nc.tensor.matmul(
                            sc[:, 512 * side:512 * (side + 1)],
                            lhsT=xT_sb[0][r0:r0 + DK, 128 * (i % 16):128 * (i % 16) + 128],
                            rhs=xT_sb[0][r0:r0 + DK, 0:512],
                            start=True, stop=True, tile_position=(r0, 0))
                    ex = expp.tile([128, 1024], BF16, name=f"mbex{i}", tag="expp")
                    with nc.allow_low_precision(reason="bench"):
                        nc.scalar.activation(ex[:], sc[:], EXPF)
                    exs2.append(ex)
                    if with_ctx and i >= LAG:
                        j = i - LAG
                        sk = j % 16
                        for side in range(2):
                            nc.tensor.matmul(
                                ctxp[side][64 * side:64 * side + 64, :],
                                lhsT=xT_sb[1][:, 64 * side:64 * side + 64],
                                rhs=exs2[j][:, 512 * side:512 * (side + 1)],
                                start=(sk == 0), stop=(sk == 15),
                                tile_position=(0, 64 * side),
                                skip_group_check=True)
                        if sk == 15:
                            cj = outp.tile([128, 512], F32, name=f"mbcu{j}",
                                           tag="ctxu")
                            nc.vector.tensor_copy(cj[0:64, :], ctxp[0][0:64, :])
                            nc.vector.tensor_copy(cj[64:128, :],
                                                  ctxp[1][64:128, :])
                            ctxp = [ps_ctx.tile([128, 512], F32,
                                                name=f"mbctx{j}_{s}",
                                                tag=f"ps_ctx{s}")
                                    for s in range(2)]
                junk = outp.tile([128, D], F32, name="junk", tag="o_sb")
                with nc.allow_low_precision(reason="bench"):
                    for ex in exs2[-4:]:
                        nc.vector.tensor_copy(junk[:, 0:8], ex[:, 0:8])
                for st in range(ST):
                    nc.sync.dma_start(out=out[128 * st:128 * (st + 1), :],
                                      in_=junk[:])

            if variant.startswith("pe"):
                # PE-only microbench: 128 steps of the kernel's per-step MM
                # mix, junk-fed, no ACT.  pe2s2c = 2 score strips + 2 ctx
                # col-strips; pe2s1c = 2 score strips + 1 merged ctx MM.
                merged = variant == "pe2s1c"
                ctxp = [ps_ctx.tile([128, 512], F32, name=f"pbctx{s}",
                                    tag=f"ps_ctx{s}") for s in range(2)]
                exj = qkv.tile([128, 1024], BF16, name="exj", tag="exj")
                with nc.allow_low_precision(reason="bench"):
                    nc.vector.tensor_copy(exj[:], xT_sb[2][:, 0:1024])
                for i in range(128):
                    sk = i % 16
                    sc = ps_sc.tile([128, 1024], F32, name=f"pbsc{i}",
                                    tag="ps_sc")
                    for side in range(2):
                        r0 = 64 * side
                        nc.tensor.matmul(
                            sc[:, 512 * side:512 * (side + 1)],
                            lhsT=xT_sb[0][r0:r0 + DK, 128 * sk:128 * sk + 128],
                            rhs=xT_sb[0][r0:r0 + DK, 0:512],
                            start=True, stop=True, tile_position=(r0, 0))
                    if merged:
                        nc.tensor.matmul(
                            ctxp[0][:, :],
                            lhsT=xT_sb[1][:, 0:128],
                            rhs=exj[:, 0:512],
                            start=(sk == 0), stop=(sk == 15),
                            skip_group_check=True)
                    else:
                        for side in range(2):
                            nc.tensor.matmul(
                                ctxp[side][64 * side:64 * side + 64, :],
                                lhsT=xT_sb[1][:, 64 * side:64 * side + 64],
                                rhs=exj[:, 512 * side:512 * (side + 1)],
                                start=(sk == 0), stop=(sk == 15),
                                tile_position=(0, 64 * side),
                                skip_group_check=True)
                    if sk == 15:
                        cj = outp.tile([128, 512], F32, name=f"pbcu{i}",
                                       tag="ctxu")
                        nc.vector.tensor_copy(cj[0:64, :], ctxp[0][0:64, :])
                        nc.vector.tensor_copy(
                            cj[64:128, :],
                            ctxp[0 if merged else 1][64:128, :])
                        ctxp = [ps_ctx.tile([128, 512], F32,
                                            name=f"pbctx{i}_{s}",
                                            tag=f"ps_ctx{s}") for s in range(2)]
                junk = outp.tile([128, D], F32, name="junk", tag="o_sb")
                with nc.allow_low_precision(reason="bench"):
                    nc.vector.tensor_copy(junk[:, 0:512], cj[:])
                    nc.vector.tensor_copy(junk[:, 512:D], cj[:, 0:D - 512])
                for st in range(ST):
                    nc.sync.dma_start(out=out[128 * st:128 * (st + 1), :],
                                      in_=junk[:])

            if variant == "pe64":
                # Uniform 64x64-tile mode: per step 4 score sub-MMs
                # (T0,T2,T8,T10) + 4 ctx sub-MMs, no mode switches.
                ctxp = [ps_ctx.tile([128, 512], F32, name=f"p6ctx{s}",
                                    tag=f"ps_ctx{s}") for s in range(2)]
                exj = qkv.tile([128, 1024], BF16, name="exj", tag="exj")
                with nc.allow_low_precision(reason="bench"):
                    nc.vector.tensor_copy(exj[:], xT_sb[2][:, 0:1024])
                for i in range(128):
                    sk = i % 16
                    sc = ps_sc.tile([128, 1024], F32, name=f"p6sc{i}",
                                    tag="ps_sc")
                    for s in range(2):        # head side = SBUF row half
                        for h in range(2):    # keys half = out partitions
                            nc.tensor.matmul(
                                sc[64 * h:64 * h + 64,
                                   512 * s:512 * s + 512],
                                lhsT=xT_sb[0][64 * s:64 * s + DK,
                                              128 * sk + 64 * h:
                                              128 * sk + 64 * h + 64],
                                rhs=xT_sb[0][64 * s:64 * s + DK, 0:512],
                                start=True, stop=True,
                                tile_position=(64 * s, 64 * h))
                    for kh in range(2):       # key half = SBUF rows
                        for s in range(2):    # head side = out partitions
                            nc.tensor.matmul(
                                ctxp[kh][64 * s:64 * s + 64, :],
                                lhsT=xT_sb[1][64 * kh:64 * kh + 64,
                                              64 * s:64 * s + 64],
                                rhs=exj[64 * kh:64 * kh + 64,
                                        512 * s:512 * s + 512],
                                start=(sk == 0), stop=(sk == 15),
                                tile_position=(64 * kh, 64 * s),
                                skip_group_check=True)
                    if sk == 15:
                        cj = outp.tile([128, 512], F32, name=f"p6cu{i}",
                                       tag="ctxu")
                        nc.vector.tensor_copy(cj[:], ctxp[0][:])
                        nc.vector.tensor_copy(cj[:, 0:256], ctxp[1][:, 0:256])
                        ctxp = [ps_ctx.tile([128, 512], F32,
                                            name=f"p6ctx{i}_{s}",
                                            tag=f"ps_ctx{s}") for s in range(2)]
                junk = outp.tile([128, D], F32, name="junk", tag="o_sb")
                with nc.allow_low_precision(reason="bench"):
                    nc.vector.tensor_copy(junk[:, 0:512], cj[:])
                    nc.vector.tensor_copy(junk[:, 512:D], cj[:, 0:D - 512])
                for st in range(ST):
                    nc.sync.dma_start(out=out[128 * st:128 * (st + 1), :],
                                      in_=junk[:])

            if variant.startswith("exp"):
                # ACT microbench suite: exp<n><kind> with kind in
                #   p: [128,1024] PSUM-f32 -> SBUF-bf16   (kernel's op)
                #   s: [128,1024] SBUF-f32 -> SBUF-bf16
                #   b: [128,1024] SBUF-bf16 -> SBUF-bf16
                #   w: [128,2048] PSUM-f32 -> SBUF-bf16   (wide)
                #   q: [128,1024] PSUM-f32 -> PSUM-f32    (psum dst)
                m = __import__("re").match(r"exp(\d+)(\w)", variant)
                n_ops, kind = int(m.group(1)), m.group(2)
                width = 2048 if kind == "w" else 1024
                if kind in ("p", "w", "q"):
                    srcs = [ps_sc.tile([128, width], F32, name=f"mb_sc{i}",
                                       tag="mb_sc0") for i in range(2)]
                    for sct in srcs:
                        for s0 in range(0, width, 512):
                            nc.tensor.matmul(
                                sct[:, s0:s0 + 512],
                                lhsT=w_sb["wk"][0][:, 0:128],
                                rhs=xT_sb[0][:, 0:512], start=True, stop=True)
                else:
                    dt = BF16 if kind == "b" else F32
                    srcs = [qkv.tile([128, width], dt, name=f"mb_sb{i}",
                                     tag=f"mb_sb{i}") for i in range(2)]
                    pst = ps_sc.tile([128, width], F32, name="mb_ps",
                                     tag="mb_sc0")
                    for s0 in range(0, width, 512):
                        nc.tensor.matmul(
                            pst[:, s0:s0 + 512], lhsT=w_sb["wk"][0][:, 0:128],
                            rhs=xT_sb[0][:, 0:512], start=True, stop=True)
                    with nc.allow_low_precision(reason="bench"):
                        for sct in srcs:
                            nc.vector.tensor_copy(sct[:], pst[:])
                if kind == "q":
                    dsts = [ps_aux.tile([128, 512], F32, name=f"mb_d{i}",
                                        tag="ps_aux") for i in range(2)]
                else:
                    dsts = None
                exs = []
                for i in range(n_ops):
                    with nc.allow_low_precision(reason="bench"):
                        if kind == "q":
                            nc.scalar.activation(dsts[i % 2][:, 0:512],
                                                 srcs[i % 2][:, 0:512], EXPF)
                        else:
                            ex = expp.tile([128, width], BF16, name=f"mbex{i}",
                                           tag="expp")
                            nc.scalar.activation(ex[:], srcs[i % 2][:], EXPF)
                            exs.append(ex)
                junk = outp.tile([128, D], F32, name="junk", tag="o_sb")
                with nc.allow_low_precision(reason="bench"):
                    if kind == "q":
                        nc.vector.tensor_copy(junk[:, 0:512], dsts[0][:])
                        nc.vector.tensor_copy(junk[:, 512:D], dsts[1][:, 0:D - 512])
                    for ex in exs[-4:]:
                        nc.vector.tensor_copy(junk[:, 0:8], ex[:, 0:8])
                for st in range(ST):
                    nc.sync.dma_start(out=out[128 * st:128 * (st + 1), :],
                                      in_=junk[:])

            if variant != "full":
                pass
            else:
                # ---------- task list ----------
                tasks = []
                seq_ctr = [0]

                def add_task(ready, deadline, cost, fn):
                    tasks.append(_Task(ready, deadline, seq_ctr[0], cost, fn))
                    seq_ctr[0] += 1

                # wk / wq chunks (except the two prologue chunks)
                for t in range(NPAIR):
                    for c in range(4):
                        if t == 0 and c == 0:
                            continue  # prologue
                        dl = 64 * t + 4 * c - 1
                        for ph in range(3):
                            add_task(0, dl, 0.45,
                                     (lambda nm, tt, cc, p: lambda:
                                      emit_qk_part(nm, tt, cc, p))("wk", t, c, ph))
                for t in range(NPAIR):
                    for q in range(NQ):
                        if t == 0 and q == 0:
                            continue  # prologue
                        dl = 64 * t + 16 * q - 1
                        for ph in range(3):
                            add_task(0, dl, 0.45,
                                     (lambda nm, tt, cc, p: lambda:
                                      emit_qk_part(nm, tt, cc, p))("wq", t, q, ph))
                for st in range(ST):
                    for ph in range(2):
                        add_task(0, st + LAG - 1, 0.35,
                                 (lambda s, p: lambda: emit_v_part(s, p))(st, ph))

                # ---------- prologue ----------
                emit_qk_chunk("wk", 0, 0)
                emit_qk_chunk("wq", 0, 0)

                # ---------- main pipeline ----------
                groups = [(p, q) for p in range(NPAIR) for q in range(NQ)]
                steps = [(gi, sk) for gi in range(len(groups))
                         for sk in range(ST)]
                ctx_ps = {}
                pending = deque()
                delayed = []  # (due_step, fn) deterministic deferred emissions
                BUDGET = 0.50

                def emit_evict(gi, cur_step):
                    pair, q = groups[gi]
                    q0 = q * QW
                    ctxu = outp.tile([128, QW], F32, name=f"ctxu{gi}",
                                     tag="ctxu")
                    nc.vector.tensor_copy(ctxu[:, :], ctx_ps[gi][:, :])
                    den = misc.tile([2, QW], F32, name=f"den{gi}", tag="den")
                    nc.sync.dma_start(out=den[0:1, :], in_=ctxu[48:49, :])
                    nc.sync.dma_start(out=den[1:2, :], in_=ctxu[112:113, :])
                    rec = misc.tile([2, QW], F32R, name=f"rec{gi}", tag="rec")
                    with nc.allow_low_precision(reason="fp32r for bcast matmul"):
                        nc.vector.reciprocal(rec[:], den[:])

                    def norm():
                        bc_ps = ps_aux.tile([128, 512], F32, name=f"bc{gi}",
                                            tag="ps_aux")
                        nc.tensor.matmul(bc_ps[:], lhsT=ones_sb[:], rhs=rec[:],
                                         start=True, stop=True)
                        with nc.allow_low_precision(reason="bf16 ctxT"):
                            nc.vector.tensor_mul(
                                ctxT_sb[pair][:, q0:q0 + QW], ctxu[:], bc_ps[:])
                        if pair == 1:
                            for sti in range(QW // 128):
                                for c0, c1 in ((0, 512), (512, D)):
                                    add_task(cur_step + 3, cur_step + 24,
                                             0.35 if c0 else 0.45,
                                             (lambda qq, ss, a, b: lambda:
                                              emit_outproj_chunk(qq, ss, a, b))(
                                                  q, sti, c0, c1))
                    delayed.append((cur_step + 2, norm))

                def emit_ctx(gi, sk, ex, cur_step):
                    pair, _ = groups[gi]
                    if sk == 0:
                        ctx_ps[gi] = ps_ctx.tile([128, QW], F32,
                                                 name=f"ctx{gi}",
                                                 tag="ps_ctx0")
                    for side in range(2):
                        nc.tensor.matmul(
                            ctx_ps[gi][64 * side:64 * side + 64, :],
                            lhsT=v_bf[sk][:, 128 * pair + 64 * side:
                                          128 * pair + 64 * side + 64],
                            rhs=ex[:, 512 * side:512 * (side + 1)],
                            start=(sk == 0), stop=(sk == ST - 1),
                            tile_position=(0, 64 * side),
                            skip_group_check=True,
                        )
                    if sk == ST - 1:
                        emit_evict(gi, cur_step)

                def emit_fillers(k):
                    spent = 0.0
                    while tasks:
                        due = [t for t in tasks if t.ready <= k]
                        if not due:
                            break
                        t = min(due, key=lambda t: (t.deadline, t.seq))
                        if t.deadline <= k or spent + t.cost <= BUDGET:
                            tasks.remove(t)
                            t.fn()
                            spent += t.cost
                        else:
                            break

                for step_idx, (gi, sk) in enumerate(steps):
                    pair, q = groups[gi]
                    q0 = q * QW
                    sc = ps_sc.tile([128, 1024], F32, name=f"sc{gi}_{sk}",
                                    tag="ps_sc")
                    for side in range(2):
                        r0 = 64 * side
                        nc.tensor.matmul(
                            sc[:, 512 * side:512 * (side + 1)],
                            lhsT=kT_sb[pair][r0:r0 + DK,
                                             128 * sk:128 * (sk + 1)],
                            rhs=qT_sb[pair][r0:r0 + DK, q0:q0 + QW],
                            start=True, stop=True,
                            tile_position=(r0, 0),
                        )
                    ex = expp.tile([128, 1024], BF16, name=f"ex{gi}_{sk}",
                                   tag="expp")
                    with nc.allow_low_precision(reason="probs in bf16"):
                        nc.scalar.activation(ex[:], sc[:], EXPF)

                    pending.append((gi, sk, ex))
                    if len(pending) > LAG:
                        cgi, csk, cex = pending.popleft()
                        emit_ctx(cgi, csk, cex, step_idx)
                    while delayed and delayed[0][0] <= step_idx:
                        delayed.pop(0)[1]()
                    emit_fillers(step_idx)

                # ---------- drain ----------
                k = len(steps)
                while pending:
                    cgi, csk, cex = pending.popleft()
                    emit_ctx(cgi, csk, cex, k)
                while delayed:
                    delayed.pop(0)[1]()
                while tasks:
                    t = min(tasks, key=lambda t: (t.ready, t.deadline, t.seq))
                    tasks.remove(t)
                    t.fn()

    nc.compile()
    return nc


def _prep_core_inputs(core, Wq, bq, Wk, bk, Wv):
    b, g = divmod(core, HPC)
    scale = 1.0 / np.sqrt(np.float32(DK))

    def pad_w(W, s):
        wp = np.zeros((D, E), np.float32)
        for j in range(HPC):
            h = HPC * g + j
            wp[:, 64 * j:64 * j + DK] = W[DK * h:DK * (h + 1), :].T * s
        return wp.astype(NPBF16)

    def pad_b(vec, s):
        bp = np.zeros((E,), np.float32)
        for j in range(HPC):
            h = HPC * g + j
            bp[64 * j:64 * j + DK] = vec[DK * h:DK * (h + 1)] * s
        return bp

    return b, {
        "wq": pad_w(Wq, scale),
        "bq": pad_b(bq, scale),
        "wk": pad_w(Wk, 1.0),
        "bk": pad_b(bk, 1.0),
        "wv": pad_w(Wv, 1.0),
    }


def _build_in_maps(x, Wq, bq, Wk, bk, Wv, Wo):
    ones2 = np.zeros((2, 128), np.float32)
    ones2[0, 0:64] = 1.0
    ones2[1, 64:128] = 1.0

    xT = [np.ascontiguousarray(x[b].T).astype(NPBF16) for b in range(B)]

    in_maps = []
    for core in range(NCORES):
        b, wmap = _prep_core_inputs(core, Wq, bq, Wk, bk, Wv)
        g = core % HPC
        wo_pad = np.zeros((E, D), np.float32)
        for j in range(HPC):
            h = HPC * g + j
            wo_pad[64 * j:64 * j + DK, :] = Wo[:, DK * h:DK * (h + 1)].T
        in_maps.append({
            "xT": xT[b],
            "wo": wo_pad.astype(NPBF16),
            "ones2": ones2,
            **wmap,
        })
    return in_maps


def _postprocess(results, Wo, bv, bo):
    const = (Wo @ bv + bo).astype(np.float32)  # folded V-bias + out bias
    out = np.empty((B, S, D), np.float32)
    for b in range(B):
        acc = results[HPC * b]["out"].astype(np.float32).copy()
        for g in range(1, HPC):
            acc += results[HPC * b + g]["out"]
        out[b] = acc + const
    return out


def get_program():
    global _PROGRAM
    if _PROGRAM is None:
        _PROGRAM = _build_program()
    return _PROGRAM


def kernel(x, Wq, bq, Wk, bk, Wv, bv, Wo, bo):
    x = np.asarray(x, np.float32)
    Wq, bq = np.asarray(Wq, np.float32), np.asarray(bq, np.float32)
    Wk, bk = np.asarray(Wk, np.float32), np.asarray(bk, np.float32)
    Wv, bv = np.asarray(Wv, np.float32), np.asarray(bv, np.float32)
    Wo, bo = np.asarray(Wo, np.float32), np.asarray(bo, np.float32)

    nc = get_program()
    in_maps = _build_in_maps(x, Wq, bq, Wk, bk, Wv, Wo)
    res = run_bass_kernel_spmd(nc, in_maps, list(range(NCORES)))
    return _postprocess(res.results, Wo, bv, bo)


# revision 5
# speedup vs baseline: 1.1959x; 1.1959x over previous
"""Multi-head attention (B=2, S=2048, D=768, H=16, dk=48) on 8 TRN2 NeuronCores.

Sharding: core c = (batch b = c//4, head-group g = c%4 of 4 heads).
Each core computes Q/K/V projections for its 4 heads, full attention over
S=2048, and a partial output projection (contribution of its heads).
Host sums the 4 partials per batch and adds the analytically-folded biases
(softmax rows sum to 1, so the V-bias contributes Wo @ bv to every row).

HW-measured engine floors per core (via loop-delta microbenches): ScalarE exp
[128,1024] = (N+352)/1.2GHz ~ 1.22us x 128 ops ~ 158us; PE matmul = stream
cols/2.4GHz + ~55ns/instr, strictly serial (tile_position pairs do NOT stream
concurrently; N>512 fp32 PSUM out is rejected by the compiler) ~ 193us.  The
kernel is therefore PE-bound; the schedule keeps both engines dense:

- software pipelining: ctx matmuls for step k are emitted LAG=4 steps later,
  so scores(k+1) is never stuck in the PE FIFO behind a ctx waiting on exp(k);
- pair-major group order (all 4 quarters of head-pair 0, then pair 1) so
  pair-1 K/Q projection chunks have late deadlines;
- projections split into <=2-matmul filler parts placed greedily (by deadline)
  into per-step PE slack;
- ctx accumulates into ONE [128,512] PSUM tile (sides at partition halves via
  col-strip tile_position) -> 1 bank, single-copy eviction, and the freed bank
  gives the projection-accumulator pool a ring of 3 (decouples the PE FIFO
  from DVE bias-add latency via the PSUM WAR chain);
- softmax-normalize broadcast matmul deferred 2 steps so it never blocks the
  PE FIFO while waiting on the reciprocal;
- V ones-column (denominator smuggling) rewritten per tile with strip-copies
  on DVE + memsets on the idle GPSIMD engine.
- PSUM: scores 2x[128,1024] (4 banks) + ctx [128,512] (1) + aux 3x[128,512].

Empirically sensitive knobs (do not "improve" without re-measuring): expp
bufs=8 (10/12 regress 220->280us), LAG=4 (5 regresses), input DMAs on the SP
ring (ACT-ring dispatch regresses ~+50us).
"""
import os
import sys
from collections import deque

import numpy as np
import ml_dtypes

for _p in ("/opt/trn_rl_repo", "/opt/pypackages"):
    if os.path.isdir(_p) and _p not in sys.path:
        sys.path.append(_p)

import concourse.bacc as bacc
import concourse.mybir as mybir
import concourse.tile as tile
from concourse.bass_utils import run_bass_kernel_spmd

F32 = mybir.dt.float32
F32R = mybir.dt.float32r
BF16 = mybir.dt.bfloat16
NPBF16 = ml_dtypes.bfloat16

B = 2
S = 2048
D = 768
H = 16
DK = 48
HPC = 4            # heads per core
NPAIR = 2          # head pairs per core
E = NPAIR * 128    # padded per-core head dim (4 heads x 64)
KT = D // 128      # 6 contraction tiles for projections
ST = S // 128      # 16 s-tiles
NQ = 4             # sq quarters
QW = S // NQ       # 512
NCORES = 8
LAG = 4            # ctx matmuls trail scores/exp by this many steps

_PROGRAM = None


class _Task:
    __slots__ = ("ready", "deadline", "seq", "cost", "fn")

    def __init__(self, ready, deadline, seq, cost, fn):
        self.ready, self.deadline, self.seq = ready, deadline, seq
        self.cost, self.fn = cost, fn


def _build_program(variant="full"):
    nc = bacc.Bacc("TRN2", target_bir_lowering=False, debug=False)

    xT = nc.dram_tensor("xT", [D, S], BF16, kind="ExternalInput")
    wq = nc.dram_tensor("wq", [D, E], BF16, kind="ExternalInput")
    wk = nc.dram_tensor("wk", [D, E], BF16, kind="ExternalInput")
    wv = nc.dram_tensor("wv", [D, E], BF16, kind="ExternalInput")
    wo = nc.dram_tensor("wo", [E, D], BF16, kind="ExternalInput")
    bq = nc.dram_tensor("bq", [E], F32, kind="ExternalInput")
    bk = nc.dram_tensor("bk", [E], F32, kind="ExternalInput")
    ones2 = nc.dram_tensor("ones2", [2, 128], F32R, kind="ExternalInput")
    out = nc.dram_tensor("out", [S, D], F32, kind="ExternalOutput")

    EXPF = mybir.ActivationFunctionType.Exp

    with tile.TileContext(nc) as tc:
        with (
            tc.tile_pool(name="xw", bufs=1) as xw,          # x + weights
            tc.tile_pool(name="qkv", bufs=1) as qkv,        # qT/kT/v/ctxT
            tc.tile_pool(name="expp", bufs=8) as expp,      # exp tiles
            tc.tile_pool(name="outp", bufs=4) as outp,      # ctxu + out staging
            tc.tile_pool(name="misc", bufs=4) as misc,      # denom/recip
            tc.tile_pool(name="ps_sc", bufs=2, space="PSUM") as ps_sc,   # 4 banks
            tc.tile_pool(name="ps_ctx", bufs=1, space="PSUM") as ps_ctx,  # 2 banks
            tc.tile_pool(name="ps_aux", bufs=3, space="PSUM") as ps_aux,  # 3 banks
        ):
            # ---------- input DMAs ----------
            # Prologue-critical tensors go on the ACT HWDGE ring: dispatched at
            # body top (ACT idles there), so across For_i iterations they
            # prefetch while the SP ring is still draining output DMAs.
            xT_sb = [xw.tile([128, S], BF16, name=f"xT_sb{k}", tag=f"xT_sb{k}")
                     for k in range(KT)]
            w_sb = {nm: [xw.tile([128, E], BF16, name=f"{nm}_sb{k}",
                                 tag=f"{nm}_sb{k}") for k in range(KT)]
                    for nm in ("wk", "wq", "wv")}
            for k in range(KT):
                nc.sync.dma_start(out=w_sb["wk"][k][:],
                                    in_=wk[128 * k:128 * (k + 1), :])
                nc.sync.dma_start(out=xT_sb[k][:, 0:512],
                                    in_=xT[128 * k:128 * (k + 1), 0:512])
                nc.sync.dma_start(out=w_sb["wq"][k][:],
                                    in_=wq[128 * k:128 * (k + 1), :])
            bias_sb = {}
            for nm, dram in (("bq", bq), ("bk", bk)):
                t = xw.tile([128, NPAIR], F32, name=f"{nm}_sb", tag=f"{nm}_sb")
                nc.sync.dma_start(out=t[:], in_=dram.rearrange("(t p) -> p t", p=128))
                bias_sb[nm] = t
            for k in range(KT):
                nc.sync.dma_start(out=w_sb["wv"][k][:],
                                    in_=wv[128 * k:128 * (k + 1), :])

            # Bulk of x + output-projection weights on the SP ring.
            for c in range(1, 4):
                for k in range(KT):
                    nc.sync.dma_start(
                        out=xT_sb[k][:, 512 * c:512 * (c + 1)],
                        in_=xT[128 * k:128 * (k + 1), 512 * c:512 * (c + 1)])
            wo_sb = []
            for k in range(NPAIR):
                t = xw.tile([128, D], BF16, name=f"wo_sb{k}", tag=f"wo_sb{k}")
                nc.sync.dma_start(out=t[:], in_=wo[128 * k:128 * (k + 1), :])
                wo_sb.append(t)
            ones_sb = xw.tile([2, 128], F32R, name="ones_sb", tag="ones_sb")
            nc.sync.dma_start(out=ones_sb[:], in_=ones2[:])

            # ---------- persistent activations (bf16) ----------
            qT_sb = [qkv.tile([128, S], BF16, name=f"qT_sb{p}", tag=f"qT_sb{p}")
                     for p in range(NPAIR)]
            kT_sb = [qkv.tile([128, S], BF16, name=f"kT_sb{p}", tag=f"kT_sb{p}")
                     for p in range(NPAIR)]
            v_bf = [qkv.tile([128, E], BF16, name=f"v_bf{st}", tag=f"v_bf{st}")
                    for st in range(ST)]
            ctxT_sb = [qkv.tile([128, S], BF16, name=f"ctxT_sb{p}", tag=f"ctxT_sb{p}")
                       for p in range(NPAIR)]

            # ---------- projection part emitters ----------
            chunk_ps = {}

            def emit_qk_part(nm, t, c, phase):
                dst = kT_sb if nm == "wk" else qT_sb
                if phase == 0:
                    chunk_ps[(nm, t, c)] = ps_aux.tile(
                        [128, 512], F32, name=f"ps_{nm}{t}_{c}", tag="ps_aux")
                ps = chunk_ps[(nm, t, c)]
                for k in range(2 * phase, 2 * phase + 2):
                    nc.tensor.matmul(
                        ps[:],
                        lhsT=w_sb[nm][k][:, 128 * t:128 * (t + 1)],
                        rhs=xT_sb[k][:, 512 * c:512 * (c + 1)],
                        start=(k == 0), stop=(k == KT - 1),
                        skip_group_check=True,
                    )
                if phase == 2:
                    bias = "bk" if nm == "wk" else "bq"
                    with nc.allow_low_precision(reason="bf16 q/k"):
                        nc.vector.tensor_scalar_add(
                            dst[t][:, 512 * c:512 * (c + 1)], ps[:],
                            bias_sb[bias][:, t:t + 1])

            def emit_qk_chunk(nm, t, c):
                for ph in range(3):
                    emit_qk_part(nm, t, c, ph)

            def emit_v_part(st, phase):
                if phase == 0:
                    chunk_ps[("v", st)] = ps_aux.tile(
                        [128, 512], F32, name=f"ps_v{st}", tag="ps_aux")
                psv = chunk_ps[("v", st)][:, 0:E]
                for k in range(3 * phase, 3 * phase + 3):
                    nc.tensor.matmul(
                        psv,
                        lhsT=xT_sb[k][:, 128 * st:128 * (st + 1)],
                        rhs=w_sb["wv"][k][:],
                        start=(k == 0), stop=(k == KT - 1),
                        skip_group_check=True,
                    )
                if phase == 1:
                    with nc.allow_low_precision(reason="probs@v in bf16"):
                        nc.vector.tensor_copy(v_bf[st][:], psv)
                    for j in range(HPC):
                        nc.vector.memset(v_bf[st][:, 64 * j + 48:64 * j + 49], 1.0)

            o_sb_tiles = {}

            def emit_outproj_chunk(q, sti, c0, c1):
                st = q * (QW // 128) + sti
                if st not in o_sb_tiles:
                    o_sb_tiles[st] = outp.tile([128, D], F32, name=f"o_sb{st}",
                                               tag="o_sb")
                o_sb = o_sb_tiles[st]
                ps = ps_aux.tile([128, 512], F32, name=f"ps_o{st}_{c0}",
                                 tag="ps_aux")
                pso = ps[:, 0:c1 - c0]
                for k in range(NPAIR):
                    nc.tensor.matmul(
                        pso,
                        lhsT=ctxT_sb[k][:, 128 * st:128 * (st + 1)],
                        rhs=wo_sb[k][:, c0:c1],
                        start=(k == 0), stop=(k == NPAIR - 1),
                    )
                nc.vector.tensor_copy(o_sb[:, c0:c1], pso)
                nc.sync.dma_start(out=out[128 * st:128 * (st + 1), c0:c1],
                                  in_=o_sb[:, c0:c1])

            if variant.startswith("scexp"):
                # Minimal scores->exp pipeline (junk data straight from xT):
                # scexp2 / scexp3 = sc ring of 2 / 3; scexpctx2 adds lagged
                # ctx matmuls like the real kernel.
                nring = int(variant[-1])
                with_ctx = "ctx" in variant
                scs, exs2 = [], []
                ctxp = None
                if with_ctx:
                    ctxp = [ps_ctx.tile([128, 512], F32, name=f"mbctx{s}",
                                        tag=f"ps_ctx{s}") for s in range(2)]
                for i in range(128):
                    sc = ps_sc.tile([128, 1024], F32, name=f"mbsc{i}",
                                    tag="ps_sc", bufs=nring)
                    for side in range(2):
                        r0 = 64 * side
                        nc.tensor.matmul(
                            sc[:, 512 * side:512 * (side + 1)],
                            lhsT=xT_sb[0][r0:r0 + DK, 128 * (i % 16):128 * (i % 16) + 128],
                            rhs=xT_sb[0][r0:r0 + DK, 0:512],
                            start=True, stop=True, tile_position=(r0, 0))
                    ex = expp.tile([128, 1024], BF16, name=f"mbex{i}", tag="expp")
                    with nc.allow_low_precision(reason="bench"):
                        nc.scalar.activation(ex[:], sc[:], EXPF)
                    exs2.append(ex)
                    if with_ctx and i >= LAG:
                        j = i - LAG
                        sk = j % 16
                        for side in range(2):
                            nc.tensor.matmul(
                                ctxp[side][64 * side:64 * side + 64, :],
                                lhsT=xT_sb[1][:, 64 * side:64 * side + 64],
                                rhs=exs2[j][:, 512 * side:512 * (side + 1)],
                                start=(sk == 0), stop=(sk == 15),
                                tile_position=(0, 64 * side),
                                skip_group_check=True)
                        if sk == 15:
                            cj = outp.tile([128, 512], F32, name=f"mbcu{j}",
                                           tag="ctxu")
                            nc.vector.tensor_copy(cj[0:64, :], ctxp[0][0:64, :])
                            nc.vector.tensor_copy(cj[64:128, :],
                                                  ctxp[1][64:128, :])
                            ctxp = [ps_ctx.tile([128, 512], F32,
                                                name=f"mbctx{j}_{s}",
                                                tag=f"ps_ctx{s}")
                                    for s in range(2)]
                junk = outp.tile([128, D], F32, name="junk", tag="o_sb")
                with nc.allow_low_precision(reason="bench"):
                    for ex in exs2[-4:]:
                        nc.vector.tensor_copy(junk[:, 0:8], ex[:, 0:8])
                for st in range(ST):
                    nc.sync.dma_start(out=out[128 * st:128 * (st + 1), :],
                                      in_=junk[:])

            if variant.startswith("pe"):
                # PE-only microbench: 128 steps of the kernel's per-step MM
                # mix, junk-fed, no ACT.  pe2s2c = 2 score strips + 2 ctx
                # col-strips; pe2s1c = 2 score strips + 1 merged ctx MM.
                merged = variant == "pe2s1c"
                ctxp = [ps_ctx.tile([128, 512], F32, name=f"pbctx{s}",
                                    tag=f"ps_ctx{s}") for s in range(2)]
                exj = qkv.tile([128, 1024], BF16, name="exj", tag="exj")
                with nc.allow_low_precision(reason="bench"):
                    nc.vector.tensor_copy(exj[:], xT_sb[2][:, 0:1024])
                for i in range(128):
                    sk = i % 16
                    sc = ps_sc.tile([128, 1024], F32, name=f"pbsc{i}",
                                    tag="ps_sc")
                    for side in range(2):
                        r0 = 64 * side
                        nc.tensor.matmul(
                            sc[:, 512 * side:512 * (side + 1)],
                            lhsT=xT_sb[0][r0:r0 + DK, 128 * sk:128 * sk + 128],
                            rhs=xT_sb[0][r0:r0 + DK, 0:512],
                            start=True, stop=True, tile_position=(r0, 0))
                    if merged:
                        nc.tensor.matmul(
                            ctxp[0][:, :],
                            lhsT=xT_sb[1][:, 0:128],
                            rhs=exj[:, 0:512],
                            start=(sk == 0), stop=(sk == 15),
                            skip_group_check=True)
                    else:
                        for side in range(2):
                            nc.tensor.matmul(
                                ctxp[side][64 * side:64 * side + 64, :],
                                lhsT=xT_sb[1][:, 64 * side:64 * side + 64],
                                rhs=exj[:, 512 * side:512 * (side + 1)],
                                start=(sk == 0), stop=(sk == 15),
                                tile_position=(0, 64 * side),
                                skip_group_check=True)
                    if sk == 15:
                        cj = outp.tile([128, 512], F32, name=f"pbcu{i}",
                                       tag="ctxu")
                        nc.vector.tensor_copy(cj[0:64, :], ctxp[0][0:64, :])
                        nc.vector.tensor_copy(
                            cj[64:128, :],
                            ctxp[0 if merged else 1][64:128, :])
                        ctxp = [ps_ctx.tile([128, 512], F32,
                                            name=f"pbctx{i}_{s}",
                                            tag=f"ps_ctx{s}") for s in range(2)]
                junk = outp.tile([128, D], F32, name="junk", tag="o_sb")
                with nc.allow_low_precision(reason="bench"):
                    nc.vector.tensor_copy(junk[:, 0:512], cj[:])
                    nc.vector.tensor_copy(junk[:, 512:D], cj[:, 0:D - 512])
                for st in range(ST):
                    nc.sync.dma_start(out=out[128 * st:128 * (st + 1), :],
                                      in_=junk[:])

            if variant == "pe64":
                # Uniform 64x64-tile mode: per step 4 score sub-MMs
                # (T0,T2,T8,T10) + 4 ctx sub-MMs, no mode switches.
                ctxp = [ps_ctx.tile([128, 512], F32, name=f"p6ctx{s}",
                                    tag=f"ps_ctx{s}") for s in range(2)]
                exj = qkv.tile([128, 1024], BF16, name="exj", tag="exj")
                with nc.allow_low_precision(reason="bench"):
                    nc.vector.tensor_copy(exj[:], xT_sb[2][:, 0:1024])
                for i in range(128):
                    sk = i % 16
                    sc = ps_sc.tile([128, 1024], F32, name=f"p6sc{i}",
                                    tag="ps_sc")
                    for s in range(2):        # head side = SBUF row half
                        for h in range(2):    # keys half = out partitions
                            nc.tensor.matmul(
                                sc[64 * h:64 * h + 64,
                                   512 * s:512 * s + 512],
                                lhsT=xT_sb[0][64 * s:64 * s + DK,
                                              128 * sk + 64 * h:
                                              128 * sk + 64 * h + 64],
                                rhs=xT_sb[0][64 * s:64 * s + DK, 0:512],
                                start=True, stop=True,
                                tile_position=(64 * s, 64 * h))
                    for kh in range(2):       # key half = SBUF rows
                        for s in range(2):    # head side = out partitions
                            nc.tensor.matmul(
                                ctxp[kh][64 * s:64 * s + 64, :],
                                lhsT=xT_sb[1][64 * kh:64 * kh + 64,
                                              64 * s:64 * s + 64],
                                rhs=exj[64 * kh:64 * kh + 64,
                                        512 * s:512 * s + 512],
                                start=(sk == 0), stop=(sk == 15),
                                tile_position=(64 * kh, 64 * s),
                                skip_group_check=True)
                    if sk == 15:
                        cj = outp.tile([128, 512], F32, name=f"p6cu{i}",
                                       tag="ctxu")
                        nc.vector.tensor_copy(cj[:], ctxp[0][:])
                        nc.vector.tensor_copy(cj[:, 0:256], ctxp[1][:, 0:256])
                        ctxp = [ps_ctx.tile([128, 512], F32,
                                            name=f"p6ctx{i}_{s}",
                                            tag=f"ps_ctx{s}") for s in range(2)]
                junk = outp.tile([128, D], F32, name="junk", tag="o_sb")
                with nc.allow_low_precision(reason="bench"):
                    nc.vector.tensor_copy(junk[:, 0:512], cj[:])
                    nc.vector.tensor_copy(junk[:, 512:D], cj[:, 0:D - 512])
                for st in range(ST):
                    nc.sync.dma_start(out=out[128 * st:128 * (st + 1), :],
                                      in_=junk[:])

            if variant.startswith("exp"):
                # ACT microbench suite: exp<n><kind> with kind in
                #   p: [128,1024] PSUM-f32 -> SBUF-bf16   (kernel's op)
                #   s: [128,1024] SBUF-f32 -> SBUF-bf16
                #   b: [128,1024] SBUF-bf16 -> SBUF-bf16
                #   w: [128,2048] PSUM-f32 -> SBUF-bf16   (wide)
                #   q: [128,1024] PSUM-f32 -> PSUM-f32    (psum dst)
                m = __import__("re").match(r"exp(\d+)(\w)", variant)
                n_ops, kind = int(m.group(1)), m.group(2)
                width = 2048 if kind == "w" else 1024
                if kind in ("p", "w", "q"):
                    srcs = [ps_sc.tile([128, width], F32, name=f"mb_sc{i}",
                                       tag="mb_sc0") for i in range(2)]
                    for sct in srcs:
                        for s0 in range(0, width, 512):
                            nc.tensor.matmul(
                                sct[:, s0:s0 + 512],
                                lhsT=w_sb["wk"][0][:, 0:128],
                                rhs=xT_sb[0][:, 0:512], start=True, stop=True)
                else:
                    dt = BF16 if kind == "b" else F32
                    srcs = [qkv.tile([128, width], dt, name=f"mb_sb{i}",
                                     tag=f"mb_sb{i}") for i in range(2)]
                    pst = ps_sc.tile([128, width], F32, name="mb_ps",
                                     tag="mb_sc0")
                    for s0 in range(0, width, 512):
                        nc.tensor.matmul(
                            pst[:, s0:s0 + 512], lhsT=w_sb["wk"][0][:, 0:128],
                            rhs=xT_sb[0][:, 0:512], start=True, stop=True)
                    with nc.allow_low_precision(reason="bench"):
                        for sct in srcs:
                            nc.vector.tensor_copy(sct[:], pst[:])
                if kind == "q":
                    dsts = [ps_aux.tile([128, 512], F32, name=f"mb_d{i}",
                                        tag="ps_aux") for i in range(2)]
                else:
                    dsts = None
                exs = []
                for i in range(n_ops):
                    with nc.allow_low_precision(reason="bench"):
                        if kind == "q":
                            nc.scalar.activation(dsts[i % 2][:, 0:512],
                                                 srcs[i % 2][:, 0:512], EXPF)
                        else:
                            ex = expp.tile([128, width], BF16, name=f"mbex{i}",
                                           tag="expp")
                            nc.scalar.activation(ex[:], srcs[i % 2][:], EXPF)
                            exs.append(ex)
                junk = outp.tile([128, D], F32, name="junk", tag="o_sb")
                with nc.allow_low_precision(reason="bench"):
                    if kind == "q":
                        nc.vector.tensor_copy(junk[:, 0:512], dsts[0][:])
                        nc.vector.tensor_copy(junk[:, 512:D], dsts[1][:, 0:D - 512])
                    for ex in exs[-4:]:
                        nc.vector.tensor_copy(junk[:, 0:8], ex[:, 0:8])
                for st in range(ST):
                    nc.sync.dma_start(out=out[128 * st:128 * (st + 1), :],
                                      in_=junk[:])

            if variant != "full":
                pass
            else:
                # ---------- task list ----------
                tasks = []
                seq_ctr = [0]

                def add_task(ready, deadline, cost, fn):
                    tasks.append(_Task(ready, deadline, seq_ctr[0], cost, fn))
                    seq_ctr[0] += 1

                # wk / wq chunks (except the two prologue chunks)
                for t in range(NPAIR):
                    for c in range(4):
                        if t == 0 and c == 0:
                            continue  # prologue
                        dl = 64 * t + 4 * c - 1
                        for ph in range(3):
                            add_task(0, dl, 0.45,
                                     (lambda nm, tt, cc, p: lambda:
                                      emit_qk_part(nm, tt, cc, p))("wk", t, c, ph))
                for t in range(NPAIR):
                    for q in range(NQ):
                        if t == 0 and q == 0:
                            continue  # prologue
                        dl = 64 * t + 16 * q - 1
                        for ph in range(3):
                            add_task(0, dl, 0.45,
                                     (lambda nm, tt, cc, p: lambda:
                                      emit_qk_part(nm, tt, cc, p))("wq", t, q, ph))
                for st in range(ST):
                    for ph in range(2):
                        add_task(0, st + LAG - 1, 0.35,
                                 (lambda s, p: lambda: emit_v_part(s, p))(st, ph))

                # ---------- prologue ----------
                emit_qk_chunk("wk", 0, 0)
                emit_qk_chunk("wq", 0, 0)

                # ---------- main pipeline ----------
                groups = [(p, q) for p in range(NPAIR) for q in range(NQ)]
                steps = [(gi, sk) for gi in range(len(groups))
                         for sk in range(ST)]
                ctx_ps = {}
                pending = deque()
                delayed = []  # (due_step, fn) deterministic deferred emissions
                BUDGET = 0.50

                def emit_evict(gi, cur_step):
                    pair, q = groups[gi]
                    q0 = q * QW
                    ctxu = outp.tile([128, QW], F32, name=f"ctxu{gi}",
                                     tag="ctxu")
                    nc.vector.tensor_copy(ctxu[:, :], ctx_ps[gi][:, :])
                    den = misc.tile([2, QW], F32, name=f"den{gi}", tag="den")
                    nc.sync.dma_start(out=den[0:1, :], in_=ctxu[48:49, :])
                    nc.sync.dma_start(out=den[1:2, :], in_=ctxu[112:113, :])
                    rec = misc.tile([2, QW], F32R, name=f"rec{gi}", tag="rec")
                    with nc.allow_low_precision(reason="fp32r for bcast matmul"):
                        nc.vector.reciprocal(rec[:], den[:])

                    def norm():
                        bc_ps = ps_aux.tile([128, 512], F32, name=f"bc{gi}",
                                            tag="ps_aux")
                        nc.tensor.matmul(bc_ps[:], lhsT=ones_sb[:], rhs=rec[:],
                                         start=True, stop=True)
                        with nc.allow_low_precision(reason="bf16 ctxT"):
                            nc.vector.tensor_mul(
                                ctxT_sb[pair][:, q0:q0 + QW], ctxu[:], bc_ps[:])
                        if pair == 1:
                            for sti in range(QW // 128):
                                for c0, c1 in ((0, 512), (512, D)):
                                    add_task(cur_step + 3, cur_step + 24,
                                             0.35 if c0 else 0.45,
                                             (lambda qq, ss, a, b: lambda:
                                              emit_outproj_chunk(qq, ss, a, b))(
                                                  q, sti, c0, c1))
                    delayed.append((cur_step + 2, norm))

                def emit_ctx(gi, sk, ex, cur_step):
                    pair, _ = groups[gi]
                    if sk == 0:
                        ctx_ps[gi] = ps_ctx.tile([128, QW], F32,
                                                 name=f"ctx{gi}",
                                                 tag="ps_ctx0")
                    for side in range(2):
                        nc.tensor.matmul(
                            ctx_ps[gi][64 * side:64 * side + 64, :],
                            lhsT=v_bf[sk][:, 128 * pair + 64 * side:
                                          128 * pair + 64 * side + 64],
                            rhs=ex[:, 512 * side:512 * (side + 1)],
                            start=(sk == 0), stop=(sk == ST - 1),
                            tile_position=(0, 64 * side),
                            skip_group_check=True,
                        )
                    if sk == ST - 1:
                        emit_evict(gi, cur_step)

                def emit_fillers(k):
                    spent = 0.0
                    while tasks:
                        due = [t for t in tasks if t.ready <= k]
                        if not due:
                            break
                        t = min(due, key=lambda t: (t.deadline, t.seq))
                        if t.deadline <= k or spent + t.cost <= BUDGET:
                            tasks.remove(t)
                            t.fn()
                            spent += t.cost
                        else:
                            break

                for step_idx, (gi, sk) in enumerate(steps):
                    pair, q = groups[gi]
                    q0 = q * QW
                    sc = ps_sc.tile([128, 1024], F32, name=f"sc{gi}_{sk}",
                                    tag="ps_sc")
                    for side in range(2):
                        r0 = 64 * side
                        nc.tensor.matmul(
                            sc[:, 512 * side:512 * (side + 1)],
                            lhsT=kT_sb[pair][r0:r0 + DK,
                                             128 * sk:128 * (sk + 1)],
                            rhs=qT_sb[pair][r0:r0 + DK, q0:q0 + QW],
                            start=True, stop=True,
                            tile_position=(r0, 0),
                        )
                    ex = expp.tile([128, 1024], BF16, name=f"ex{gi}_{sk}",
                                   tag="expp")
                    with nc.allow_low_precision(reason="probs in bf16"):
                        nc.scalar.activation(ex[:], sc[:], EXPF)

                    pending.append((gi, sk, ex))
                    if len(pending) > LAG:
                        cgi, csk, cex = pending.popleft()
                        emit_ctx(cgi, csk, cex, step_idx)
                    while delayed and delayed[0][0] <= step_idx:
                        delayed.pop(0)[1]()
                    emit_fillers(step_idx)

                # ---------- drain ----------
                k = len(steps)
                while pending:
                    cgi, csk, cex = pending.popleft()
                    emit_ctx(cgi, csk, cex, k)
                while delayed:
                    delayed.pop(0)[1]()
                while tasks:
                    t = min(tasks, key=lambda t: (t.ready, t.deadline, t.seq))
                    tasks.remove(t)
                    t.fn()

    nc.compile()
    return nc


def _prep_core_inputs(core, Wq, bq, Wk, bk, Wv):
    b, g = divmod(core, HPC)
    scale = 1.0 / np.sqrt(np.float32(DK))

    def pad_w(W, s):
        wp = np.zeros((D, E), np.float32)
        for j in range(HPC):
            h = HPC * g + j
            wp[:, 64 * j:64 * j + DK] = W[DK * h:DK * (h + 1), :].T * s
        return wp.astype(NPBF16)

    def pad_b(vec, s):
        bp = np.zeros((E,), np.float32)
        for j in range(HPC):
            h = HPC * g + j
            bp[64 * j:64 * j + DK] = vec[DK * h:DK * (h + 1)] * s
        return bp

    return b, {
        "wq": pad_w(Wq, scale),
        "bq": pad_b(bq, scale),
        "wk": pad_w(Wk, 1.0),
        "bk": pad_b(bk, 1.0),
        "wv": pad_w(Wv, 1.0),
    }


def _build_in_maps(x, Wq, bq, Wk, bk, Wv, Wo):
    ones2 = np.zeros((2, 128), np.float32)
    ones2[0, 0:64] = 1.0
    ones2[1, 64:128] = 1.0

    xT = [np.ascontiguousarray(x[b].T).astype(NPBF16) for b in range(B)]

    in_maps = []
    for core in range(NCORES):
        b, wmap = _prep_core_inputs(core, Wq, bq, Wk, bk, Wv)
        g = core % HPC
        wo_pad = np.zeros((E, D), np.float32)
        for j in range(HPC):
            h = HPC * g + j
            wo_pad[64 * j:64 * j + DK, :] = Wo[:, DK * h:DK * (h + 1)].T
        in_maps.append({
            "xT": xT[b],
            "wo": wo_pad.astype(NPBF16),
            "ones2": ones2,
            **wmap,
        })
    return in_maps


def _postprocess(results, Wo, bv, bo):
    const = (Wo @ bv + bo).astype(np.float32)  # folded V-bias + out bias
    out = np.empty((B, S, D), np.float32)
    for b in range(B):
        acc = results[HPC * b]["out"].astype(np.float32).copy()
        for g in range(1, HPC):
            acc += results[HPC * b + g]["out"]
        out[b] = acc + const
    return out


def get_program():
    global _PROGRAM
    if _PROGRAM is None:
        _PROGRAM = _build_program()
    return _PROGRAM


def kernel(x, Wq, bq, Wk, bk, Wv, bv, Wo, bo):
    x = np.asarray(x, np.float32)
    Wq, bq = np.asarray(Wq, np.float32), np.asarray(bq, np.float32)
    Wk, bk = np.asarray(Wk, np.float32), np.asarray(bk, np.float32)
    Wv, bv = np.asarray(Wv, np.float32), np.asarray(bv, np.float32)
    Wo, bo = np.asarray(Wo, np.float32), np.asarray(bo, np.float32)

    nc = get_program()
    in_maps = _build_in_maps(x, Wq, bq, Wk, bk, Wv, Wo)
    res = run_bass_kernel_spmd(nc, in_maps, list(range(NCORES)))
    return _postprocess(res.results, Wo, bv, bo)


# revision 6
# speedup vs baseline: 1.6318x; 1.3644x over previous
"""Multi-head attention (B=2, S=2048, D=768, H=16, dk=48) on 8 TRN2 NeuronCores.

Sharding: core c = (batch b = c//4, head-group g = c%4 of 4 heads).
Each core computes Q/K/V projections for its 4 heads, full attention over
S=2048, and a partial output projection (contribution of its heads).
Host sums the 4 partials per batch and adds the analytically-folded biases
(softmax rows sum to 1, so the V-bias contributes Wo @ bv to every row).

HW-measured engine floors per core (via loop-delta microbenches): ScalarE exp
[128,1024] = (N+352)/1.2GHz ~ 1.22us x 128 ops ~ 158us; PE matmul = stream
cols/2.4GHz + ~55ns/instr, strictly serial (tile_position pairs do NOT stream
concurrently; N>512 fp32 PSUM out is rejected by the compiler) ~ 193us.  The
kernel is therefore PE-bound; the schedule keeps both engines dense:

- software pipelining: ctx matmuls for step k are emitted LAG=4 steps later,
  so scores(k+1) is never stuck in the PE FIFO behind a ctx waiting on exp(k);
- pair-major group order (all 4 quarters of head-pair 0, then pair 1) so
  pair-1 K/Q projection chunks have late deadlines;
- projections split into <=2-matmul filler parts placed greedily (by deadline)
  into per-step PE slack;
- ctx accumulates into ONE [128,512] PSUM tile (sides at partition halves via
  col-strip tile_position) -> 1 bank, single-copy eviction, and the freed bank
  gives the projection-accumulator pool a ring of 3 (decouples the PE FIFO
  from DVE bias-add latency via the PSUM WAR chain);
- softmax-normalize broadcast matmul deferred 2 steps so it never blocks the
  PE FIFO while waiting on the reciprocal;
- V ones-column (denominator smuggling): full-tile PSUM->SBUF copy (keeps the
  zero padding initialized -- strip-copies leave NaN garbage that poisons the
  out-projection MACs) + per-strip DVE memsets.
- PSUM: scores 2x[128,1024] (4 banks) + ctx [128,512] (1) + aux 3x[128,512].

Empirically sensitive knobs (do not "improve" without re-measuring): expp
bufs=8 (10/12 regress 220->280us), LAG=4 (5 regresses), input DMAs on the SP
ring (ACT-ring dispatch regresses ~+50us).
"""
import os
import sys
from collections import deque

import numpy as np
import ml_dtypes

for _p in ("/opt/trn_rl_repo", "/opt/pypackages"):
    if os.path.isdir(_p) and _p not in sys.path:
        sys.path.append(_p)

import concourse.bacc as bacc
import concourse.mybir as mybir
import concourse.tile as tile
from concourse.bass_utils import run_bass_kernel_spmd

F32 = mybir.dt.float32
F32R = mybir.dt.float32r
BF16 = mybir.dt.bfloat16
NPBF16 = ml_dtypes.bfloat16

B = 2
S = 2048
D = 768
H = 16
DK = 48
HPC = 4            # heads per core
NPAIR = 2          # head pairs per core
E = NPAIR * 128    # padded per-core head dim (4 heads x 64)
KT = D // 128      # 6 contraction tiles for projections
ST = S // 128      # 16 s-tiles
NQ = 4             # sq quarters
QW = S // NQ       # 512
NCORES = 8
LAG = 4            # ctx matmuls trail scores/exp by this many steps

_PROGRAM = None


class _Task:
    __slots__ = ("ready", "deadline", "seq", "cost", "fn")

    def __init__(self, ready, deadline, seq, cost, fn):
        self.ready, self.deadline, self.seq = ready, deadline, seq
        self.cost, self.fn = cost, fn


def _build_program(variant="full"):
    nc = bacc.Bacc("TRN2", target_bir_lowering=False, debug=False)

    xT = nc.dram_tensor("xT", [D, S], BF16, kind="ExternalInput")
    wq = nc.dram_tensor("wq", [D, E], BF16, kind="ExternalInput")
    wk = nc.dram_tensor("wk", [D, E], BF16, kind="ExternalInput")
    wv = nc.dram_tensor("wv", [D, E], BF16, kind="ExternalInput")
    wo = nc.dram_tensor("wo", [E, D], BF16, kind="ExternalInput")
    bq = nc.dram_tensor("bq", [E], F32, kind="ExternalInput")
    bk = nc.dram_tensor("bk", [E], F32, kind="ExternalInput")
    ones2 = nc.dram_tensor("ones2", [2, 128], F32R, kind="ExternalInput")
    out = nc.dram_tensor("out", [S, D], F32, kind="ExternalOutput")

    EXPF = mybir.ActivationFunctionType.Exp

    with tile.TileContext(nc) as tc:
        with (
            tc.tile_pool(name="xw", bufs=1) as xw,          # x + weights
            tc.tile_pool(name="qkv", bufs=1) as qkv,        # qT/kT/v/ctxT
            tc.tile_pool(name="expp", bufs=8) as expp,      # exp tiles
            tc.tile_pool(name="outp", bufs=4) as outp,      # ctxu + out staging
            tc.tile_pool(name="misc", bufs=4) as misc,      # denom/recip
            tc.tile_pool(name="ps_sc", bufs=2, space="PSUM") as ps_sc,   # 4 banks
            tc.tile_pool(name="ps_ctx", bufs=1, space="PSUM") as ps_ctx,  # 2 banks
            tc.tile_pool(name="ps_aux", bufs=3, space="PSUM") as ps_aux,  # 3 banks
        ):
            # ---------- input DMAs ----------
            # Prologue-critical tensors go on the ACT HWDGE ring: dispatched at
            # body top (ACT idles there), so across For_i iterations they
            # prefetch while the SP ring is still draining output DMAs.
            xT_sb = [xw.tile([128, S], BF16, name=f"xT_sb{k}", tag=f"xT_sb{k}")
                     for k in range(KT)]
            w_sb = {nm: [xw.tile([128, E], BF16, name=f"{nm}_sb{k}",
                                 tag=f"{nm}_sb{k}") for k in range(KT)]
                    for nm in ("wk", "wq", "wv")}
            for k in range(KT):
                nc.sync.dma_start(out=w_sb["wk"][k][:],
                                    in_=wk[128 * k:128 * (k + 1), :])
                nc.sync.dma_start(out=xT_sb[k][:, 0:512],
                                    in_=xT[128 * k:128 * (k + 1), 0:512])
                nc.sync.dma_start(out=w_sb["wq"][k][:],
                                    in_=wq[128 * k:128 * (k + 1), :])
            bias_sb = {}
            for nm, dram in (("bq", bq), ("bk", bk)):
                t = xw.tile([128, NPAIR], F32, name=f"{nm}_sb", tag=f"{nm}_sb")
                nc.sync.dma_start(out=t[:], in_=dram.rearrange("(t p) -> p t", p=128))
                bias_sb[nm] = t
            for k in range(KT):
                nc.sync.dma_start(out=w_sb["wv"][k][:],
                                    in_=wv[128 * k:128 * (k + 1), :])

            # Bulk of x + output-projection weights on the SP ring.
            for c in range(1, 4):
                for k in range(KT):
                    nc.sync.dma_start(
                        out=xT_sb[k][:, 512 * c:512 * (c + 1)],
                        in_=xT[128 * k:128 * (k + 1), 512 * c:512 * (c + 1)])
            wo_sb = []
            for k in range(NPAIR):
                t = xw.tile([128, D], BF16, name=f"wo_sb{k}", tag=f"wo_sb{k}")
                nc.sync.dma_start(out=t[:], in_=wo[128 * k:128 * (k + 1), :])
                wo_sb.append(t)
            ones_sb = xw.tile([2, 128], F32R, name="ones_sb", tag="ones_sb")
            nc.sync.dma_start(out=ones_sb[:], in_=ones2[:])

            # ---------- persistent activations (bf16) ----------
            qT_sb = [qkv.tile([128, S], BF16, name=f"qT_sb{p}", tag=f"qT_sb{p}")
                     for p in range(NPAIR)]
            kT_sb = [qkv.tile([128, S], BF16, name=f"kT_sb{p}", tag=f"kT_sb{p}")
                     for p in range(NPAIR)]
            v_bf = [qkv.tile([128, E], BF16, name=f"v_bf{st}", tag=f"v_bf{st}")
                    for st in range(ST)]
            ctxT_sb = [qkv.tile([128, S], BF16, name=f"ctxT_sb{p}", tag=f"ctxT_sb{p}")
                       for p in range(NPAIR)]

            # ---------- projection part emitters ----------
            chunk_ps = {}

            def emit_qk_part(nm, t, c, phase):
                dst = kT_sb if nm == "wk" else qT_sb
                if phase == 0:
                    chunk_ps[(nm, t, c)] = ps_aux.tile(
                        [128, 512], F32, name=f"ps_{nm}{t}_{c}", tag="ps_aux")
                ps = chunk_ps[(nm, t, c)]
                for k in range(2 * phase, 2 * phase + 2):
                    nc.tensor.matmul(
                        ps[:],
                        lhsT=w_sb[nm][k][:, 128 * t:128 * (t + 1)],
                        rhs=xT_sb[k][:, 512 * c:512 * (c + 1)],
                        start=(k == 0), stop=(k == KT - 1),
                        skip_group_check=True,
                    )
                if phase == 2:
                    bias = "bk" if nm == "wk" else "bq"
                    with nc.allow_low_precision(reason="bf16 q/k"):
                        nc.vector.tensor_scalar_add(
                            dst[t][:, 512 * c:512 * (c + 1)], ps[:],
                            bias_sb[bias][:, t:t + 1])

            def emit_qk_chunk(nm, t, c):
                for ph in range(3):
                    emit_qk_part(nm, t, c, ph)

            def emit_v_part(st, phase):
                if phase == 0:
                    chunk_ps[("v", st)] = ps_aux.tile(
                        [128, 512], F32, name=f"ps_v{st}", tag="ps_aux")
                psv = chunk_ps[("v", st)][:, 0:E]
                for k in range(3 * phase, 3 * phase + 3):
                    nc.tensor.matmul(
                        psv,
                        lhsT=xT_sb[k][:, 128 * st:128 * (st + 1)],
                        rhs=w_sb["wv"][k][:],
                        start=(k == 0), stop=(k == KT - 1),
                        skip_group_check=True,
                    )
                if phase == 1:
                    with nc.allow_low_precision(reason="probs@v in bf16"):
                        nc.vector.tensor_copy(v_bf[st][:], psv)
                    for j in range(HPC):
                        nc.vector.memset(v_bf[st][:, 64 * j + 48:64 * j + 49], 1.0)

            o_sb_tiles = {}

            def emit_outproj_chunk(q, sti, c0, c1):
                st = q * (QW // 128) + sti
                if st not in o_sb_tiles:
                    o_sb_tiles[st] = outp.tile([128, D], F32, name=f"o_sb{st}",
                                               tag="o_sb")
                o_sb = o_sb_tiles[st]
                ps = ps_aux.tile([128, 512], F32, name=f"ps_o{st}_{c0}",
                                 tag="ps_aux")
                pso = ps[:, 0:c1 - c0]
                for k in range(NPAIR):
                    nc.tensor.matmul(
                        pso,
                        lhsT=ctxT_sb[k][:, 128 * st:128 * (st + 1)],
                        rhs=wo_sb[k][:, c0:c1],
                        start=(k == 0), stop=(k == NPAIR - 1),
                    )
                nc.vector.tensor_copy(o_sb[:, c0:c1], pso)
                nc.sync.dma_start(out=out[128 * st:128 * (st + 1), c0:c1],
                                  in_=o_sb[:, c0:c1])

            if variant.startswith("scexp"):
                # Minimal scores->exp pipeline (junk data straight from xT):
                # scexp2 / scexp3 = sc ring of 2 / 3; scexpctx2 adds lagged
                # ctx matmuls like the real kernel.
                nring = int(variant[-1])
                with_ctx = "ctx" in variant
                scs, exs2 = [], []
                ctxp = None
                if with_ctx:
                    ctxp = [ps_ctx.tile([128, 512], F32, name=f"mbctx{s}",
                                        tag=f"ps_ctx{s}") for s in range(2)]
                for i in range(128):
                    sc = ps_sc.tile([128, 1024], F32, name=f"mbsc{i}",
                                    tag="ps_sc", bufs=nring)
                    for side in range(2):
                        r0 = 64 * side
                        nc.tensor.matmul(
                            sc[:, 512 * side:512 * (side + 1)],
                            lhsT=xT_sb[0][r0:r0 + DK, 128 * (i % 16):128 * (i % 16) + 128],
                            rhs=xT_sb[0][r0:r0 + DK, 0:512],
                            start=True, stop=True, tile_position=(r0, 0))
                    ex = expp.tile([128, 1024], BF16, name=f"mbex{i}", tag="expp")
                    with nc.allow_low_precision(reason="bench"):
                        nc.scalar.activation(ex[:], sc[:], EXPF)
                    exs2.append(ex)
                    if with_ctx and i >= LAG:
                        j = i - LAG
                        sk = j % 16
                        for side in range(2):
                            nc.tensor.matmul(
                                ctxp[side][64 * side:64 * side + 64, :],
                                lhsT=xT_sb[1][:, 64 * side:64 * side + 64],
                                rhs=exs2[j][:, 512 * side:512 * (side + 1)],
                                start=(sk == 0), stop=(sk == 15),
                                tile_position=(0, 64 * side),
                                skip_group_check=True)
                        if sk == 15:
                            cj = outp.tile([128, 512], F32, name=f"mbcu{j}",
                                           tag="ctxu")
                            nc.vector.tensor_copy(cj[0:64, :], ctxp[0][0:64, :])
                            nc.vector.tensor_copy(cj[64:128, :],
                                                  ctxp[1][64:128, :])
                            ctxp = [ps_ctx.tile([128, 512], F32,
                                                name=f"mbctx{j}_{s}",
                                                tag=f"ps_ctx{s}")
                                    for s in range(2)]
                junk = outp.tile([128, D], F32, name="junk", tag="o_sb")
                with nc.allow_low_precision(reason="bench"):
                    for ex in exs2[-4:]:
                        nc.vector.tensor_copy(junk[:, 0:8], ex[:, 0:8])
                for st in range(ST):
                    nc.sync.dma_start(out=out[128 * st:128 * (st + 1), :],
                                      in_=junk[:])

            if variant.startswith("pe"):
                # PE-only microbench: 128 steps of the kernel's per-step MM
                # mix, junk-fed, no ACT.  pe2s2c = 2 score strips + 2 ctx
                # col-strips; pe2s1c = 2 score strips + 1 merged ctx MM.
                merged = variant == "pe2s1c"
                ctxp = [ps_ctx.tile([128, 512], F32, name=f"pbctx{s}",
                                    tag=f"ps_ctx{s}") for s in range(2)]
                exj = qkv.tile([128, 1024], BF16, name="exj", tag="exj")
                with nc.allow_low_precision(reason="bench"):
                    nc.vector.tensor_copy(exj[:], xT_sb[2][:, 0:1024])
                for i in range(128):
                    sk = i % 16
                    sc = ps_sc.tile([128, 1024], F32, name=f"pbsc{i}",
                                    tag="ps_sc")
                    for side in range(2):
                        r0 = 64 * side
                        nc.tensor.matmul(
                            sc[:, 512 * side:512 * (side + 1)],
                            lhsT=xT_sb[0][r0:r0 + DK, 128 * sk:128 * sk + 128],
                            rhs=xT_sb[0][r0:r0 + DK, 0:512],
                            start=True, stop=True, tile_position=(r0, 0))
                    if merged:
                        nc.tensor.matmul(
                            ctxp[0][:, :],
                            lhsT=xT_sb[1][:, 0:128],
                            rhs=exj[:, 0:512],
                            start=(sk == 0), stop=(sk == 15),
                            skip_group_check=True)
                    else:
                        for side in range(2):
                            nc.tensor.matmul(
                                ctxp[side][64 * side:64 * side + 64, :],
                                lhsT=xT_sb[1][:, 64 * side:64 * side + 64],
                                rhs=exj[:, 512 * side:512 * (side + 1)],
                                start=(sk == 0), stop=(sk == 15),
                                tile_position=(0, 64 * side),
                                skip_group_check=True)
                    if sk == 15:
                        cj = outp.tile([128, 512], F32, name=f"pbcu{i}",
                                       tag="ctxu")
                        nc.vector.tensor_copy(cj[0:64, :], ctxp[0][0:64, :])
                        nc.vector.tensor_copy(
                            cj[64:128, :],
                            ctxp[0 if merged else 1][64:128, :])
                        ctxp = [ps_ctx.tile([128, 512], F32,
                                            name=f"pbctx{i}_{s}",
                                            tag=f"ps_ctx{s}") for s in range(2)]
                junk = outp.tile([128, D], F32, name="junk", tag="o_sb")
                with nc.allow_low_precision(reason="bench"):
                    nc.vector.tensor_copy(junk[:, 0:512], cj[:])
                    nc.vector.tensor_copy(junk[:, 512:D], cj[:, 0:D - 512])
                for st in range(ST):
                    nc.sync.dma_start(out=out[128 * st:128 * (st + 1), :],
                                      in_=junk[:])

            if variant == "pe64":
                # Uniform 64x64-tile mode: per step 4 score sub-MMs
                # (T0,T2,T8,T10) + 4 ctx sub-MMs, no mode switches.
                ctxp = [ps_ctx.tile([128, 512], F32, name=f"p6ctx{s}",
                                    tag=f"ps_ctx{s}") for s in range(2)]
                exj = qkv.tile([128, 1024], BF16, name="exj", tag="exj")
                with nc.allow_low_precision(reason="bench"):
                    nc.vector.tensor_copy(exj[:], xT_sb[2][:, 0:1024])
                for i in range(128):
                    sk = i % 16
                    sc = ps_sc.tile([128, 1024], F32, name=f"p6sc{i}",
                                    tag="ps_sc")
                    for s in range(2):        # head side = SBUF row half
                        for h in range(2):    # keys half = out partitions
                            nc.tensor.matmul(
                                sc[64 * h:64 * h + 64,
                                   512 * s:512 * s + 512],
                                lhsT=xT_sb[0][64 * s:64 * s + DK,
                                              128 * sk + 64 * h:
                                              128 * sk + 64 * h + 64],
                                rhs=xT_sb[0][64 * s:64 * s + DK, 0:512],
                                start=True, stop=True,
                                tile_position=(64 * s, 64 * h))
                    for kh in range(2):       # key half = SBUF rows
                        for s in range(2):    # head side = out partitions
                            nc.tensor.matmul(
                                ctxp[kh][64 * s:64 * s + 64, :],
                                lhsT=xT_sb[1][64 * kh:64 * kh + 64,
                                              64 * s:64 * s + 64],
                                rhs=exj[64 * kh:64 * kh + 64,
                                        512 * s:512 * s + 512],
                                start=(sk == 0), stop=(sk == 15),
                                tile_position=(64 * kh, 64 * s),
                                skip_group_check=True)
                    if sk == 15:
                        cj = outp.tile([128, 512], F32, name=f"p6cu{i}",
                                       tag="ctxu")
                        nc.vector.tensor_copy(cj[:], ctxp[0][:])
                        nc.vector.tensor_copy(cj[:, 0:256], ctxp[1][:, 0:256])
                        ctxp = [ps_ctx.tile([128, 512], F32,
                                            name=f"p6ctx{i}_{s}",
                                            tag=f"ps_ctx{s}") for s in range(2)]
                junk = outp.tile([128, D], F32, name="junk", tag="o_sb")
                with nc.allow_low_precision(reason="bench"):
                    nc.vector.tensor_copy(junk[:, 0:512], cj[:])
                    nc.vector.tensor_copy(junk[:, 512:D], cj[:, 0:D - 512])
                for st in range(ST):
                    nc.sync.dma_start(out=out[128 * st:128 * (st + 1), :],
                                      in_=junk[:])

            if variant.startswith("exp"):
                # ACT microbench suite: exp<n><kind> with kind in
                #   p: [128,1024] PSUM-f32 -> SBUF-bf16   (kernel's op)
                #   s: [128,1024] SBUF-f32 -> SBUF-bf16
                #   b: [128,1024] SBUF-bf16 -> SBUF-bf16
                #   w: [128,2048] PSUM-f32 -> SBUF-bf16   (wide)
                #   q: [128,1024] PSUM-f32 -> PSUM-f32    (psum dst)
                m = __import__("re").match(r"exp(\d+)(\w)", variant)
                n_ops, kind = int(m.group(1)), m.group(2)
                width = 2048 if kind == "w" else 1024
                if kind in ("p", "w", "q"):
                    srcs = [ps_sc.tile([128, width], F32, name=f"mb_sc{i}",
                                       tag="mb_sc0") for i in range(2)]
                    for sct in srcs:
                        for s0 in range(0, width, 512):
                            nc.tensor.matmul(
                                sct[:, s0:s0 + 512],
                                lhsT=w_sb["wk"][0][:, 0:128],
                                rhs=xT_sb[0][:, 0:512], start=True, stop=True)
                else:
                    dt = BF16 if kind == "b" else F32
                    srcs = [qkv.tile([128, width], dt, name=f"mb_sb{i}",
                                     tag=f"mb_sb{i}") for i in range(2)]
                    pst = ps_sc.tile([128, width], F32, name="mb_ps",
                                     tag="mb_sc0")
                    for s0 in range(0, width, 512):
                        nc.tensor.matmul(
                            pst[:, s0:s0 + 512], lhsT=w_sb["wk"][0][:, 0:128],
                            rhs=xT_sb[0][:, 0:512], start=True, stop=True)
                    with nc.allow_low_precision(reason="bench"):
                        for sct in srcs:
                            nc.vector.tensor_copy(sct[:], pst[:])
                if kind == "q":
                    dsts = [ps_aux.tile([128, 512], F32, name=f"mb_d{i}",
                                        tag="ps_aux") for i in range(2)]
                else:
                    dsts = None
                exs = []
                for i in range(n_ops):
                    with nc.allow_low_precision(reason="bench"):
                        if kind == "q":
                            nc.scalar.activation(dsts[i % 2][:, 0:512],
                                                 srcs[i % 2][:, 0:512], EXPF)
                        else:
                            ex = expp.tile([128, width], BF16, name=f"mbex{i}",
                                           tag="expp")
                            nc.scalar.activation(ex[:], srcs[i % 2][:], EXPF)
                            exs.append(ex)
                junk = outp.tile([128, D], F32, name="junk", tag="o_sb")
                with nc.allow_low_precision(reason="bench"):
                    if kind == "q":
                        nc.vector.tensor_copy(junk[:, 0:512], dsts[0][:])
                        nc.vector.tensor_copy(junk[:, 512:D], dsts[1][:, 0:D - 512])
                    for ex in exs[-4:]:
                        nc.vector.tensor_copy(junk[:, 0:8], ex[:, 0:8])
                for st in range(ST):
                    nc.sync.dma_start(out=out[128 * st:128 * (st + 1), :],
                                      in_=junk[:])

            if variant != "full":
                pass
            else:
                # ---------- task list ----------
                tasks = []
                seq_ctr = [0]

                def add_task(ready, deadline, cost, fn):
                    tasks.append(_Task(ready, deadline, seq_ctr[0], cost, fn))
                    seq_ctr[0] += 1

                # wk / wq chunks (except the two prologue chunks)
                for t in range(NPAIR):
                    for c in range(4):
                        if t == 0 and c == 0:
                            continue  # prologue
                        dl = 64 * t + 4 * c - 1
                        for ph in range(3):
                            add_task(0, dl, 0.45,
                                     (lambda nm, tt, cc, p: lambda:
                                      emit_qk_part(nm, tt, cc, p))("wk", t, c, ph))
                for t in range(NPAIR):
                    for q in range(NQ):
                        if t == 0 and q == 0:
                            continue  # prologue
                        dl = 64 * t + 16 * q - 1
                        for ph in range(3):
                            add_task(0, dl, 0.45,
                                     (lambda nm, tt, cc, p: lambda:
                                      emit_qk_part(nm, tt, cc, p))("wq", t, q, ph))
                for st in range(ST):
                    for ph in range(2):
                        add_task(0, st + LAG - 1, 0.35,
                                 (lambda s, p: lambda: emit_v_part(s, p))(st, ph))

                # ---------- prologue ----------
                emit_qk_chunk("wk", 0, 0)
                emit_qk_chunk("wq", 0, 0)

                # ---------- main pipeline ----------
                groups = [(p, q) for p in range(NPAIR) for q in range(NQ)]
                steps = [(gi, sk) for gi in range(len(groups))
                         for sk in range(ST)]
                ctx_ps = {}
                pending = deque()
                delayed = []  # (due_step, fn) deterministic deferred emissions
                BUDGET = 0.50

                def emit_evict(gi, cur_step):
                    pair, q = groups[gi]
                    q0 = q * QW
                    ctxu = outp.tile([128, QW], F32, name=f"ctxu{gi}",
                                     tag="ctxu")
                    nc.vector.tensor_copy(ctxu[:, :], ctx_ps[gi][:, :])
                    den = misc.tile([2, QW], F32, name=f"den{gi}", tag="den")
                    nc.sync.dma_start(out=den[0:1, :], in_=ctxu[48:49, :])
                    nc.sync.dma_start(out=den[1:2, :], in_=ctxu[112:113, :])
                    rec = misc.tile([2, QW], F32R, name=f"rec{gi}", tag="rec")
                    with nc.allow_low_precision(reason="fp32r for bcast matmul"):
                        nc.vector.reciprocal(rec[:], den[:])

                    def norm():
                        bc_ps = ps_aux.tile([128, 512], F32, name=f"bc{gi}",
                                            tag="ps_aux")
                        nc.tensor.matmul(bc_ps[:], lhsT=ones_sb[:], rhs=rec[:],
                                         start=True, stop=True)
                        with nc.allow_low_precision(reason="bf16 ctxT"):
                            nc.vector.tensor_mul(
                                ctxT_sb[pair][:, q0:q0 + QW], ctxu[:], bc_ps[:])
                        if pair == 1:
                            for sti in range(QW // 128):
                                for c0, c1 in ((0, 512), (512, D)):
                                    add_task(cur_step + 3, cur_step + 24,
                                             0.35 if c0 else 0.45,
                                             (lambda qq, ss, a, b: lambda:
                                              emit_outproj_chunk(qq, ss, a, b))(
                                                  q, sti, c0, c1))
                    delayed.append((cur_step + 2, norm))

                def emit_ctx(gi, sk, ex, cur_step):
                    pair, _ = groups[gi]
                    if sk == 0:
                        ctx_ps[gi] = ps_ctx.tile([128, QW], F32,
                                                 name=f"ctx{gi}",
                                                 tag="ps_ctx0")
                    for side in range(2):
                        nc.tensor.matmul(
                            ctx_ps[gi][64 * side:64 * side + 64, :],
                            lhsT=v_bf[sk][:, 128 * pair + 64 * side:
                                          128 * pair + 64 * side + 64],
                            rhs=ex[:, 512 * side:512 * (side + 1)],
                            start=(sk == 0), stop=(sk == ST - 1),
                            tile_position=(0, 64 * side),
                            skip_group_check=True,
                        )
                    if sk == ST - 1:
                        emit_evict(gi, cur_step)

                def emit_fillers(k):
                    spent = 0.0
                    while tasks:
                        due = [t for t in tasks if t.ready <= k]
                        if not due:
                            break
                        t = min(due, key=lambda t: (t.deadline, t.seq))
                        if t.deadline <= k or spent + t.cost <= BUDGET:
                            tasks.remove(t)
                            t.fn()
                            spent += t.cost
                        else:
                            break

                for step_idx, (gi, sk) in enumerate(steps):
                    pair, q = groups[gi]
                    q0 = q * QW
                    sc = ps_sc.tile([128, 1024], F32, name=f"sc{gi}_{sk}",
                                    tag="ps_sc")
                    for side in range(2):
                        r0 = 64 * side
                        nc.tensor.matmul(
                            sc[:, 512 * side:512 * (side + 1)],
                            lhsT=kT_sb[pair][r0:r0 + DK,
                                             128 * sk:128 * (sk + 1)],
                            rhs=qT_sb[pair][r0:r0 + DK, q0:q0 + QW],
                            start=True, stop=True,
                            tile_position=(r0, 0),
                        )
                    ex = expp.tile([128, 1024], BF16, name=f"ex{gi}_{sk}",
                                   tag="expp")
                    with nc.allow_low_precision(reason="probs in bf16"):
                        nc.scalar.activation(ex[:], sc[:], EXPF)

                    pending.append((gi, sk, ex))
                    if len(pending) > LAG:
                        cgi, csk, cex = pending.popleft()
                        emit_ctx(cgi, csk, cex, step_idx)
                    while delayed and delayed[0][0] <= step_idx:
                        delayed.pop(0)[1]()
                    emit_fillers(step_idx)

                # ---------- drain ----------
                k = len(steps)
                while pending:
                    cgi, csk, cex = pending.popleft()
                    emit_ctx(cgi, csk, cex, k)
                while delayed:
                    delayed.pop(0)[1]()
                while tasks:
                    t = min(tasks, key=lambda t: (t.ready, t.deadline, t.seq))
                    tasks.remove(t)
                    t.fn()

    nc.compile()
    return nc


def _prep_core_inputs(core, Wq, bq, Wk, bk, Wv):
    b, g = divmod(core, HPC)
    scale = 1.0 / np.sqrt(np.float32(DK))

    def pad_w(W, s):
        wp = np.zeros((D, E), np.float32)
        for j in range(HPC):
            h = HPC * g + j
            wp[:, 64 * j:64 * j + DK] = W[DK * h:DK * (h + 1), :].T * s
        return wp.astype(NPBF16)

    def pad_b(vec, s):
        bp = np.zeros((E,), np.float32)
        for j in range(HPC):
            h = HPC * g + j
            bp[64 * j:64 * j + DK] = vec[DK * h:DK * (h + 1)] * s
        return bp

    return b, {
        "wq": pad_w(Wq, scale),
        "bq": pad_b(bq, scale),
        "wk": pad_w(Wk, 1.0),
        "bk": pad_b(bk, 1.0),
        "wv": pad_w(Wv, 1.0),
    }


def _build_in_maps(x, Wq, bq, Wk, bk, Wv, Wo):
    ones2 = np.zeros((2, 128), np.float32)
    ones2[0, 0:64] = 1.0
    ones2[1, 64:128] = 1.0

    xT = [np.ascontiguousarray(x[b].T).astype(NPBF16) for b in range(B)]

    in_maps = []
    for core in range(NCORES):
        b, wmap = _prep_core_inputs(core, Wq, bq, Wk, bk, Wv)
        g = core % HPC
        wo_pad = np.zeros((E, D), np.float32)
        for j in range(HPC):
            h = HPC * g + j
            wo_pad[64 * j:64 * j + DK, :] = Wo[:, DK * h:DK * (h + 1)].T
        in_maps.append({
            "xT": xT[b],
            "wo": wo_pad.astype(NPBF16),
            "ones2": ones2,
            **wmap,
        })
    return in_maps


def _postprocess(results, Wo, bv, bo):
    const = (Wo @ bv + bo).astype(np.float32)  # folded V-bias + out bias
    out = np.empty((B, S, D), np.float32)
    for b in range(B):
        acc = results[HPC * b]["out"].astype(np.float32).copy()
        for g in range(1, HPC):
            acc += results[HPC * b + g]["out"]
        out[b] = acc + const
    return out


def get_program():
    global _PROGRAM
    if _PROGRAM is None:
        _PROGRAM = _build_program()
    return _PROGRAM


def kernel(x, Wq, bq, Wk, bk, Wv, bv, Wo, bo):
    x = np.asarray(x, np.float32)
    Wq, bq = np.asarray(Wq, np.float32), np.asarray(bq, np.float32)
    Wk, bk = np.asarray(Wk, np.float32), np.asarray(bk, np.float32)
    Wv, bv = np.asarray(Wv, np.float32), np.asarray(bv, np.float32)
    Wo, bo = np.asarray(Wo, np.float32), np.asarray(bo, np.float32)

    nc = get_program()
    in_maps = _build_in_maps(x, Wq, bq, Wk, bk, Wv, Wo)
    res = run_bass_kernel_spmd(nc, in_maps, list(range(NCORES)))
    return _postprocess(res.results, Wo, bv, bo)
